# revision 20
# baseline (speedup 1.0000x reference)
"""BitLinear (B=8) TRN2 kernel — single-pass fp8 DoubleRow + rank-1 correction.

Math (reference):
    gamma = max(max|x|, 1e-5);  xq = clip(round(x*256/gamma), -256, 255)
    beta  = max(mean|W|, 1e-5); wq = (|W| > 0.5*beta)  in {0,1}
    y     = (xq @ wq.T) * (beta*gamma/256)

Scheme: u = x*(16/gamma) in [-16,16];  a = e4m3(u)  (one fp8 DoubleRow
pass, 2 k-tiles per instruction);  residual e = u - a is corrected by the
rank-1 term  (sum_k e[t,k]) * (colsum(wq)[o] / I), folded into the matmul
as one extra DoubleRow contraction step whose stationary row is
E8[t] = fp8(sum_k B[t,k]) (partition 0 only) and whose moving row is
s8[o] = fp8(colsum(wq)[o]/4096).  Measured rel err ~1.5e-2 (gate 2e-2).

Pipeline design (engine-FIFO aware):
  - wb (beta partial) streams in 2-ktile DMAs; gamma+beta partials ride
    ONE fused AllGather.
  - wt stream: DMA (sync q) -> abs (ACT) -> is_gt (DVE).  The DMA+abs
    run bufs-ahead from t=0; only is_gt waits on beta.
  - blocks 0-1 of x are quantized on DVE *before* the W stream in DVE
    program order (gated on gamma only), so tt0's accumulation matmuls
    trail the w8 stream as k-tiles arrive; blocks 2+ quantize on ACT
    (emitted after the paced wt-abs ops, by which time the stream has
    drained).
  - psum is split into two half-tiles (3 banks each); each half's
    rank-1 + copy happens independently so the copy of one half hides
    under the other half's matmuls (no per-tt PE bubble).
  - psS (colsum of wq) is sectioned through the pse psum slot inside
    tt0, after the kk-loop, with each half's rank-1 gated only on the
    sections it reads.

Distribution: 2x4 grid (token halves x out-feature quarters), x shipped
host-transposed so the contraction lands on partitions with no on-device
transpose; gamma/beta via per-core disjoint partials + one AllGather.

A post-compile pass drops InstLdweights whose weights AP equals the
previous load on the PE stream.
"""

import numpy as np

# ---- problem constants (hardcoded; kernel.py must be self-contained) ----
B_DIM, S_DIM, I_DIM, O_DIM = 4, 2048, 4096, 11008
N_CORES = 8
TOK_HALVES = 2
O_QUARTERS = 4
TOK_TOTAL = B_DIM * S_DIM
TOK = TOK_TOTAL // TOK_HALVES       # 4096 tokens per core
O_SH = O_DIM // O_QUARTERS          # 2752 out features per core

EPS = 1e-5


def _half_chunks(width):
    """256-wide chunks that never cross a 512-f32 psum region boundary;
    yields (off, w, region_start, region_end)."""
    chunks = []
    off = 0
    while off < width:
        rem = width - off
        w = min(256, rem)
        # absorb a short tail into one wider chunk if it stays in-region
        if rem <= 512 - (off % 512):
            w = rem
        rs = off % 512 == 0
        re = (off + w) % 512 == 0 or off + w == width
        chunks.append((off, w, rs, re))
        off += w
    return chunks


def build_kernel(I=I_DIM, TOK=TOK, O_SH=O_SH, n_cores=N_CORES,
                 tok_halves=TOK_HALVES, o_quarters=O_QUARTERS,
                 n_total=None, blk=256):
    """Per-core: xT [I, TOK] f32, wt [I, O_SH] f32, xg/wb partial slices.
    Output: y [TOK, O_SH] bf16."""
    import concourse.bacc as bacc
    import concourse.mybir as mybir
    import concourse.tile as tile
    from concourse import bass_isa
    from concourse.bass import ts

    if n_total is None:
        n_total = float(I) * float(O_SH * o_quarters)

    f32 = mybir.dt.float32
    bf16 = mybir.dt.bfloat16
    fp8 = mybir.dt.float8e4
    Alu = mybir.AluOpType
    Act = mybir.ActivationFunctionType
    DR = mybir.MatmulPerfMode.DoubleRow

    KT = I // 128
    KK = KT // 2                    # DoubleRow k-pair steps
    KQ = min(8, KT)                 # k-tiles per quantize step
    NQ = KT // KQ
    NBLK = TOK // blk
    TPB = blk // 128
    GKT = (I // o_quarters) // 128
    WKT = (I // tok_halves) // 128
    WB2 = 1                         # k-tiles per beta DMA
    inv_I = float(np.float32(1.0) / np.float32(I))

    OH = O_SH // 2                  # evacuation half width
    hchunks = _half_chunks(OH)      # within-half (off, w, rs, re)
    # psS sections (512-wide over full O_SH); h0's rank-1 needs only the
    # sections overlapping [0, OH)
    secs = []
    off = 0
    while off < O_SH:
        sw = min(512, O_SH - off)
        secs.append((off, sw))
        off += sw
    secs_h0 = [s for s in secs if s[0] < OH]
    secs_h1 = [s for s in secs if s[0] >= OH]

    nc = bacc.Bacc("TRN2", target_bir_lowering=False, debug=False,
                   num_devices=n_cores)

    xT_d = nc.dram_tensor("xT", [I, TOK], f32, kind="ExternalInput")
    wt_d = nc.dram_tensor("wt", [I, O_SH], f32, kind="ExternalInput")
    xg_d = nc.dram_tensor("xg", [I // o_quarters, TOK], f32,
                          kind="ExternalInput")
    wb_d = nc.dram_tensor("wb", [I // tok_halves, O_SH], f32,
                          kind="ExternalInput")
    y_d = nc.dram_tensor("y", [TOK, O_SH], bf16, kind="ExternalOutput")
    shared = "Shared" if n_cores > 4 else "Local"
    ccx_in = nc.dram_tensor("ccx_in", [2], f32)
    ccx_out = nc.dram_tensor("ccx_out", [2 * n_cores], f32,
                             addr_space=shared)

    xT_r = xT_d.ap().rearrange("(kt p) m -> p kt m", p=128)
    wt_r = wt_d.ap().rearrange("(kt p) o -> p kt o", p=128)
    xg_r = xg_d.ap().rearrange("(kt p) m -> p kt m", p=128)
    wb_r = wb_d.ap().rearrange("(kt p) o -> p kt o", p=128)

    with tile.TileContext(nc) as tc:
        with (
            tc.tile_pool(name="wtp", bufs=4) as wt_pool,
            tc.tile_pool(name="wbp", bufs=2) as wb_pool,
            tc.tile_pool(name="xs", bufs=2) as x_pool,
            tc.tile_pool(name="ab", bufs=2) as ab_pool,
            tc.tile_pool(name="wres", bufs=1) as wres_pool,
            tc.tile_pool(name="stat", bufs=1) as stat_pool,
            tc.tile_pool(name="yout", bufs=1) as y_pool,
            tc.tile_pool(name="ps", bufs=1, space="PSUM") as ps_pool,
            tc.tile_pool(name="pse", bufs=2, space="PSUM") as pse_pool,
        ):
            w8 = wres_pool.tile([128, KT, O_SH], fp8)       # wq in {0,1}
            wx8 = wres_pool.tile([128, 2, O_SH], fp8)       # ext weights row
            ones8 = stat_pool.tile([128, 2, 16], fp8)
            gchunk = min(TOK, 2048)
            gsub = TOK // gchunk
            gmax1 = stat_pool.tile([1, GKT * gsub], f32)
            redg1 = stat_pool.tile([1, 1], f32)
            wsum = stat_pool.tile([128, WKT // WB2], f32)
            redgw = stat_pool.tile([128, 2], f32)           # [gamma, beta]
            scx1 = stat_pool.tile([1, 2 * n_cores], f32)
            scx = stat_pool.tile([128, 2 * n_cores], f32)
            scal = stat_pool.tile([128, 8], f32)
            n16 = stat_pool.tile([128, 1], f32)
            aextA = stat_pool.tile([128, 2, blk], fp8)      # ext activations
            aextB = aextA
            redw = stat_pool.tile([128, 1], f32)

            nc.vector.memset(wx8.rearrange("p a o -> p (a o)"), 0.0)
            nc.vector.memset(aextA.rearrange("p a m -> p (a m)"), 0.0)
            nc.vector.memset(ones8.rearrange("p a b -> p (a b)"), 1.0)
            ones_lhs = ones8[:, :, 0:1]

            # ---- gamma partial: max|xg| (DVE; xg on scalar queue) ----
            for i in range(GKT):
                for j in range(gsub):
                    gx_t = x_pool.tile([128, gchunk], f32, tag="x_t",
                                       name="gx_t")
                    nc.gpsimd.dma_start(gx_t, xg_r[:, i, ts(j, gchunk)])
                    nc.gpsimd.tensor_reduce(
                        gmax1[0:1, i * gsub + j:i * gsub + j + 1], gx_t,
                        axis=mybir.AxisListType.XYZWC,
                        op=Alu.max, apply_absolute_value=True)
            nc.gpsimd.tensor_reduce(redg1, gmax1,
                                    axis=mybir.AxisListType.XYZWC,
                                    op=Alu.max)

            # ---- beta partial: sum|wb| in 2-ktile strides (ACT abs) ----
            for i in range(WKT // WB2):
                wb_t = wb_pool.tile([128, WB2, O_SH], f32, tag="wbtile",
                                    name="wb_t")
                nc.sync.dma_start(wb_t, wb_r[:, ts(i, WB2), :])
                nc.vector.tensor_reduce(
                    wsum[:, i:i + 1],
                    wb_t.rearrange("p a o -> p (a o)"),
                    axis=mybir.AxisListType.X, op=Alu.add,
                    apply_absolute_value=True)
            nc.vector.tensor_reduce(redw, wsum,
                                    axis=mybir.AxisListType.X, op=Alu.add)

            # ---- fused cross-partition + cross-core reduction ----
            nc.gpsimd.partition_all_reduce(redgw[:, 1:2], redw, channels=128,
                                           reduce_op=bass_isa.ReduceOp.add)
            nc.gpsimd.dma_start(ccx_in[0:1], redg1[0:1, 0:1])
            nc.gpsimd.dma_start(ccx_in[1:2], redgw[0:1, 1:2])
            nc.gpsimd.collective_compute(
                "AllGather", Alu.bypass,
                replica_groups=[list(range(n_cores))],
                ins=[ccx_in.ap()], outs=[ccx_out.ap()])
            nc.gpsimd.dma_start(
                scx1, ccx_out.ap().rearrange("(a b) -> a b", a=1))
            nc.gpsimd.partition_broadcast(scx, scx1)
            scx_v = scx.rearrange("p (c s) -> p s c", s=2)

            # gamma = max over cores; s16 = 16/gamma
            nc.vector.tensor_reduce(scal[:, 0:1], scx_v[:, 0, :],
                                    axis=mybir.AxisListType.X, op=Alu.max)
            nc.vector.tensor_scalar_max(scal[:, 0:1], scal[:, 0:1], EPS)
            nc.vector.reciprocal(n16, scal[:, 0:1])
            nc.vector.tensor_scalar_mul(scal[:, 3:4], n16, 16.0)

            # beta = sum over cores / n_total; thr = beta/2;
            # c_out = beta*gamma/16
            nc.vector.tensor_reduce(scal[:, 1:2], scx_v[:, 1, :],
                                    axis=mybir.AxisListType.X, op=Alu.add)
            inv_n = float(np.float32(1.0) / np.float32(n_total))
            nc.vector.tensor_scalar_mul(scal[:, 2:3], scal[:, 1:2], inv_n)
            nc.vector.tensor_scalar_max(scal[:, 2:3], scal[:, 2:3], EPS)
            nc.vector.tensor_scalar_mul(scal[:, 4:5], scal[:, 2:3], 0.5)
            nc.vector.tensor_tensor(scal[:, 6:7], scal[:, 4:5],
                                    scal[:, 4:5], op=Alu.mult)
            nc.vector.tensor_tensor(scal[:, 5:6], scal[:, 2:3],
                                    scal[:, 0:1], op=Alu.mult)
            nc.vector.tensor_scalar_mul(scal[:, 5:6], scal[:, 5:6],
                                        1.0 / 16.0)

            ab_tiles = {}

            def quantize_block(b, on_act):
                ab8 = ab_pool.tile([128, KT, 2, blk], fp8, name="ab8")
                ab_tiles[b] = ab8
                for qq in range(NQ):
                    x_t = x_pool.tile([128, KQ, blk], f32, tag="x_t",
                                      name="x_t")
                    nc.gpsimd.dma_start(
                        x_t, xT_r[:, ts(qq, KQ), ts(b, blk)])
                    a_sl = ab8[:, ts(qq, KQ), 0, :]
                    if on_act:
                        nc.scalar.activation(x_t, x_t, Act.Copy,
                                             scale=scal[:, 3:4])
                        nc.scalar.activation(a_sl, x_t, Act.Copy)
                    else:
                        nc.vector.tensor_scalar_mul(x_t, x_t,
                                                    scal[:, 3:4])
                        nc.vector.tensor_scalar_mul(a_sl, x_t, 1.0)
                    nc.vector.tensor_tensor(
                        ab8[:, ts(qq, KQ), 1, :], x_t, a_sl,
                        op=Alu.subtract)

            # block 0 quantizes on DVE, ahead of the W stream in DVE
            # program order (gated on gamma only -> PE trails the w8
            # stream through tt0); later blocks go on ACT, pre-emitted
            # a block ahead.
            quantize_block(0, on_act=False)

            # ---- W quantize stream ----
            for k in range(KT):
                wt_t = wt_pool.tile([128, O_SH], f32, tag="wtile",
                                    name="wq_t")
                nc.sync.dma_start(wt_t, wt_r[:, k, :])
                nc.vector.tensor_tensor(wt_t, wt_t, wt_t, op=Alu.mult)
                nc.vector.tensor_scalar(w8[:, k, :], wt_t, scal[:, 6:7],
                                        None, op0=Alu.is_gt)

            def ps_sections(sec_list):
                for (soff, sw) in sec_list:
                    psS = pse_pool.tile([128, 512], f32,
                                        tag="pset", name="psS")
                    for kk in range(KK):
                        nc.tensor.matmul(
                            psS[0:1, 0:sw], ones_lhs,
                            w8[:, 2 * kk:2 * kk + 2, soff:soff + sw],
                            start=(kk == 0), stop=(kk == KK - 1),
                            perf_mode=DR)
                    nc.vector.tensor_scalar_mul(
                        wx8[0:1, 0, soff:soff + sw],
                        psS[0:1, 0:sw], inv_I)

            # ---- main loop ----
            for b in range(NBLK):
                aext = aextA if b % 2 == 0 else aextB
                if b not in ab_tiles:
                    quantize_block(b, on_act=True)
                ab8 = ab_tiles[b]

                for tt in range(TPB):
                    if tt == 1 and 1 <= b + 1 < NBLK \
                            and b + 1 not in ab_tiles:
                        # pre-quantize next block here: its ACT/DVE ops
                        # land between tt0's and tt1's copies in those
                        # FIFOs, hiding under tt1's matmuls
                        quantize_block(b + 1, on_act=True)
                    first = (b == 0 and tt == 0)
                    ph = [ps_pool.tile([128, OH], f32, tag=f"ph{h}",
                                       name=f"ph{h}") for h in range(2)]
                    for kk in range(KK):
                        lhsT = ab8[:, 2 * kk:2 * kk + 2, 0, ts(tt, 128)]
                        for h in range(2):
                            base = h * OH
                            for (off, w_, rs, re) in hchunks:
                                nc.tensor.matmul(
                                    ph[h][:, off:off + w_], lhsT,
                                    w8[:, 2 * kk:2 * kk + 2,
                                       base + off:base + off + w_],
                                    start=(kk == 0 and rs), stop=False,
                                    perf_mode=DR)
                    if tt == 0:
                        # E[t] = sum_k B[t,k] -> aext fp8 row
                        psE = pse_pool.tile([128, 512], f32, tag="pset",
                                            name="psE")
                        for kk in range(KK):
                            nc.tensor.matmul(
                                psE[0:1, 0:blk], ones_lhs,
                                ab8[:, 2 * kk:2 * kk + 2, 1, :],
                                start=(kk == 0), stop=(kk == KK - 1),
                                perf_mode=DR)
                        nc.vector.tensor_scalar_mul(aext[0:1, 0, :],
                                                    psE[0:1, 0:blk], 1.0)

                    for h in range(2):
                        if first:
                            ps_sections(secs_h0 if h == 0 else secs_h1)
                        base = h * OH
                        for (off, w_, rs, re) in hchunks:
                            nc.tensor.matmul(
                                ph[h][:, off:off + w_],
                                aext[:, :, ts(tt, 128)],
                                wx8[:, :, base + off:base + off + w_],
                                start=False, stop=re, perf_mode=DR)
                        y_t = y_pool.tile([128, OH], bf16, tag="yh",
                                          name="y_t")
                        if h == 0:
                            nc.scalar.activation(y_t, ph[h], Act.Copy,
                                                 scale=scal[:, 5:6])
                        else:
                            nc.vector.tensor_scalar(y_t, ph[h],
                                                    scal[:, 5:6], None,
                                                    op0=Alu.mult)
                        nc.gpsimd.dma_start(
                            y_d.ap()[ts(b * TPB + tt, 128),
                                     base:base + OH], y_t)

    nc.compile()
    _dedup_ldweights(nc)
    return nc


def _dedup_ldweights(nc):
    """Drop InstLdweights whose weights AP equals the previous PE load."""
    removed = kept_sync = 0
    for fn in nc.m.functions:
        for blk_ in fn.blocks:
            insts = blk_.instructions
            prev_sig = None
            kill = []
            for j, ins in enumerate(insts):
                tn = type(ins).__name__
                if tn == "InstLdweights":
                    sig = (str(ins.ins[0]), str(ins.perf_mode),
                           str(ins.is_transpose))
                    if sig == prev_sig:
                        if not ins.has_wait() and not ins.has_update():
                            kill.append(j)
                        else:
                            kept_sync += 1
                    prev_sig = sig
                elif tn == "InstMatmult":
                    if ins.is_transpose:
                        prev_sig = None
            for j in reversed(kill):
                del insts[j]
            removed += len(kill)
    if removed:
        print(f"[kernel_sp] deduped {removed} redundant ldweights "
              f"({kept_sync} kept for sync)")


_CACHED_NC = None


def _get_nc():
    global _CACHED_NC
    if _CACHED_NC is None:
        _CACHED_NC = build_kernel()
    return _CACHED_NC


def shard_inputs(x, weight):
    x2 = x.reshape(TOK_TOTAL, I_DIM).astype(np.float32, copy=False)
    weight = weight.astype(np.float32, copy=False)
    xT_halves = [
        np.ascontiguousarray(x2[h * TOK:(h + 1) * TOK].T)
        for h in range(TOK_HALVES)
    ]
    wt_quarters = [
        np.ascontiguousarray(weight[q * O_SH:(q + 1) * O_SH].T)
        for q in range(O_QUARTERS)
    ]
    gk = I_DIM // O_QUARTERS
    bk = I_DIM // TOK_HALVES
    in_maps = []
    for c in range(N_CORES):
        h, q = c // O_QUARTERS, c % O_QUARTERS
        in_maps.append({
            "xT": xT_halves[h],
            "wt": wt_quarters[q],
            "xg": np.ascontiguousarray(xT_halves[h][q * gk:(q + 1) * gk]),
            "wb": np.ascontiguousarray(wt_quarters[q][h * bk:(h + 1) * bk]),
        })
    return in_maps


def unshard_output(results):
    rows = []
    for h in range(TOK_HALVES):
        cols = [np.asarray(results[h * O_QUARTERS + q]["y"])
                for q in range(O_QUARTERS)]
        rows.append(np.concatenate(cols, axis=1))
    y = np.concatenate(rows, axis=0).astype(np.float32)
    return y.reshape(B_DIM, S_DIM, O_DIM)


def run_on_cores(x, weight, trace=False):
    from concourse.bass_utils import run_bass_kernel_spmd
    nc = _get_nc()
    in_maps = shard_inputs(x, weight)
    res = run_bass_kernel_spmd(nc, in_maps, core_ids=list(range(N_CORES)),
                               trace=trace)
    return res


def kernel(x, weight):
    res = run_on_cores(x, weight, trace=False)
    return unshard_output(res.results)


# revision 21
# speedup vs baseline: 1.0052x; 1.0052x over previous
"""BitLinear (B=8) TRN2 kernel — single-pass fp8 DoubleRow + rank-1 correction.

Math (reference):
    gamma = max(max|x|, 1e-5);  xq = clip(round(x*256/gamma), -256, 255)
    beta  = max(mean|W|, 1e-5); wq = (|W| > 0.5*beta)  in {0,1}
    y     = (xq @ wq.T) * (beta*gamma/256)

Scheme: u = x*(16/gamma) in [-16,16];  a = e4m3(u)  (one fp8 DoubleRow
pass, 2 k-tiles per instruction);  residual e = u - a is corrected by the
rank-1 term  (sum_k e[t,k]) * (colsum(wq)[o] / I), folded into the matmul
as one extra DoubleRow contraction step whose stationary row is
E8[t] = fp8(sum_k B[t,k]) (partition 0 only) and whose moving row is
s8[o] = fp8(colsum(wq)[o]/4096).  Measured rel err ~1.5e-2 (gate 2e-2).

Pipeline design (engine-FIFO aware):
  - wb (beta partial) streams in 2-ktile DMAs; gamma+beta partials ride
    ONE fused AllGather.
  - wt stream: DMA (sync q) -> abs (ACT) -> is_gt (DVE).  The DMA+abs
    run bufs-ahead from t=0; only is_gt waits on beta.
  - blocks 0-1 of x are quantized on DVE *before* the W stream in DVE
    program order (gated on gamma only), so tt0's accumulation matmuls
    trail the w8 stream as k-tiles arrive; blocks 2+ quantize on ACT
    (emitted after the paced wt-abs ops, by which time the stream has
    drained).
  - psum is split into two half-tiles (3 banks each); each half's
    rank-1 + copy happens independently so the copy of one half hides
    under the other half's matmuls (no per-tt PE bubble).
  - psS (colsum of wq) is sectioned through the pse psum slot inside
    tt0, after the kk-loop, with each half's rank-1 gated only on the
    sections it reads.

Distribution: 2x4 grid (token halves x out-feature quarters), x shipped
host-transposed so the contraction lands on partitions with no on-device
transpose; gamma/beta via per-core disjoint partials + one AllGather.

A post-compile pass drops InstLdweights whose weights AP equals the
previous load on the PE stream.
"""

import numpy as np

# ---- problem constants (hardcoded; kernel.py must be self-contained) ----
B_DIM, S_DIM, I_DIM, O_DIM = 4, 2048, 4096, 11008
N_CORES = 8
TOK_HALVES = 2
O_QUARTERS = 4
TOK_TOTAL = B_DIM * S_DIM
TOK = TOK_TOTAL // TOK_HALVES       # 4096 tokens per core
O_SH = O_DIM // O_QUARTERS          # 2752 out features per core

EPS = 1e-5


def _half_chunks(width):
    """256-wide chunks that never cross a 512-f32 psum region boundary;
    yields (off, w, region_start, region_end)."""
    chunks = []
    off = 0
    while off < width:
        rem = width - off
        w = min(256, rem)
        # absorb a short tail into one wider chunk if it stays in-region
        if rem <= 512 - (off % 512):
            w = rem
        rs = off % 512 == 0
        re = (off + w) % 512 == 0 or off + w == width
        chunks.append((off, w, rs, re))
        off += w
    return chunks


def build_kernel(I=I_DIM, TOK=TOK, O_SH=O_SH, n_cores=N_CORES,
                 tok_halves=TOK_HALVES, o_quarters=O_QUARTERS,
                 n_total=None, blk=256):
    """Per-core: xT [I, TOK] f32, wt [I, O_SH] f32, xg/wb partial slices.
    Output: y [TOK, O_SH] bf16."""
    import concourse.bacc as bacc
    import concourse.mybir as mybir
    import concourse.tile as tile
    from concourse import bass_isa
    from concourse.bass import ts

    if n_total is None:
        n_total = float(I) * float(O_SH * o_quarters)

    f32 = mybir.dt.float32
    bf16 = mybir.dt.bfloat16
    fp8 = mybir.dt.float8e4
    Alu = mybir.AluOpType
    Act = mybir.ActivationFunctionType
    DR = mybir.MatmulPerfMode.DoubleRow

    KT = I // 128
    KK = KT // 2                    # DoubleRow k-pair steps
    KQ = min(8, KT)                 # k-tiles per quantize step
    NQ = KT // KQ
    NBLK = TOK // blk
    TPB = blk // 128
    GKT = (I // o_quarters) // 128
    WKT = (I // tok_halves) // 128
    WB2 = 1                         # k-tiles per beta DMA
    inv_I = float(np.float32(1.0) / np.float32(I))

    OH = O_SH // 2                  # evacuation half width
    hchunks = _half_chunks(OH)      # within-half (off, w, rs, re)
    # psS sections (512-wide over full O_SH); h0's rank-1 needs only the
    # sections overlapping [0, OH)
    secs = []
    off = 0
    while off < O_SH:
        sw = min(512, O_SH - off)
        secs.append((off, sw))
        off += sw
    secs_h0 = [s for s in secs if s[0] < OH]
    secs_h1 = [s for s in secs if s[0] >= OH]

    nc = bacc.Bacc("TRN2", target_bir_lowering=False, debug=False,
                   num_devices=n_cores)

    xT_d = nc.dram_tensor("xT", [I, TOK], f32, kind="ExternalInput")
    wt_d = nc.dram_tensor("wt", [I, O_SH], f32, kind="ExternalInput")
    xg_d = nc.dram_tensor("xg", [I // o_quarters, TOK], f32,
                          kind="ExternalInput")
    wb_d = nc.dram_tensor("wb", [I // tok_halves, O_SH], f32,
                          kind="ExternalInput")
    y_d = nc.dram_tensor("y", [TOK, O_SH], bf16, kind="ExternalOutput")
    shared = "Shared" if n_cores > 4 else "Local"
    ccx_in = nc.dram_tensor("ccx_in", [2], f32)
    ccx_out = nc.dram_tensor("ccx_out", [2 * n_cores], f32,
                             addr_space=shared)

    xT_r = xT_d.ap().rearrange("(kt p) m -> p kt m", p=128)
    wt_r = wt_d.ap().rearrange("(kt p) o -> p kt o", p=128)
    xg_r = xg_d.ap().rearrange("(kt p) m -> p kt m", p=128)
    wb_r = wb_d.ap().rearrange("(kt p) o -> p kt o", p=128)

    with tile.TileContext(nc) as tc:
        with (
            tc.tile_pool(name="wtp", bufs=4) as wt_pool,
            tc.tile_pool(name="wbp", bufs=2) as wb_pool,
            tc.tile_pool(name="xs", bufs=2) as x_pool,
            tc.tile_pool(name="ab", bufs=2) as ab_pool,
            tc.tile_pool(name="wres", bufs=1) as wres_pool,
            tc.tile_pool(name="stat", bufs=1) as stat_pool,
            tc.tile_pool(name="yout", bufs=1) as y_pool,
            tc.tile_pool(name="ps", bufs=1, space="PSUM") as ps_pool,
            tc.tile_pool(name="pse", bufs=2, space="PSUM") as pse_pool,
        ):
            w8 = wres_pool.tile([128, KT, O_SH], fp8)       # wq in {0,1}
            wx8 = wres_pool.tile([128, 2, O_SH], fp8)       # ext weights row
            ones8 = stat_pool.tile([128, 2, 16], fp8)
            gchunk = min(TOK, 2048)
            gsub = TOK // gchunk
            gmax1 = stat_pool.tile([1, GKT * gsub], f32)
            redg1 = stat_pool.tile([1, 1], f32)
            wsum = stat_pool.tile([128, WKT // WB2], f32)
            redgw = stat_pool.tile([128, 2], f32)           # [gamma, beta]
            scx1 = stat_pool.tile([1, 2 * n_cores], f32)
            scx = stat_pool.tile([128, 2 * n_cores], f32)
            scal = stat_pool.tile([128, 8], f32)
            n16 = stat_pool.tile([128, 1], f32)
            aextA = stat_pool.tile([128, 2, blk], fp8)      # ext activations
            aextB = aextA
            redw = stat_pool.tile([128, 1], f32)

            nc.vector.memset(wx8.rearrange("p a o -> p (a o)"), 0.0)
            nc.vector.memset(aextA.rearrange("p a m -> p (a m)"), 0.0)
            nc.vector.memset(ones8.rearrange("p a b -> p (a b)"), 1.0)
            ones_lhs = ones8[:, :, 0:1]

            # ---- gamma partial: max|xg| (DVE; xg on scalar queue) ----
            for i in range(GKT):
                for j in range(gsub):
                    gx_t = x_pool.tile([128, gchunk], f32, tag="x_t",
                                       name="gx_t")
                    nc.gpsimd.dma_start(gx_t, xg_r[:, i, ts(j, gchunk)])
                    nc.gpsimd.tensor_reduce(
                        gmax1[0:1, i * gsub + j:i * gsub + j + 1], gx_t,
                        axis=mybir.AxisListType.XYZWC,
                        op=Alu.max, apply_absolute_value=True)
            nc.gpsimd.tensor_reduce(redg1, gmax1,
                                    axis=mybir.AxisListType.XYZWC,
                                    op=Alu.max)

            # ---- beta partial: sum|wb| on TWO independent chains:
            # even tiles sync-q DMA -> DVE abs-add reduce, odd tiles
            # scalar-q DMA -> ACT abs+accum.  The two wbp buffers become
            # parallel self-paced pipelines (halves the pacing latency).
            for i in range(WKT // WB2):
                wb_t = wb_pool.tile([128, WB2, O_SH], f32, tag="wbtile",
                                    name="wb_t")
                if i % 2 == 0:
                    nc.sync.dma_start(wb_t, wb_r[:, ts(i, WB2), :])
                    nc.vector.tensor_reduce(
                        wsum[:, i:i + 1],
                        wb_t.rearrange("p a o -> p (a o)"),
                        axis=mybir.AxisListType.X, op=Alu.add,
                        apply_absolute_value=True)
                else:
                    nc.scalar.dma_start(wb_t, wb_r[:, ts(i, WB2), :])
                    nc.scalar.activation(
                        wb_t.rearrange("p a o -> p (a o)"),
                        wb_t.rearrange("p a o -> p (a o)"), Act.Abs,
                        accum_out=wsum[:, i:i + 1])
            nc.vector.tensor_reduce(redw, wsum,
                                    axis=mybir.AxisListType.X, op=Alu.add)

            # ---- fused cross-partition + cross-core reduction ----
            nc.gpsimd.partition_all_reduce(redgw[:, 1:2], redw, channels=128,
                                           reduce_op=bass_isa.ReduceOp.add)
            nc.gpsimd.dma_start(ccx_in[0:1], redg1[0:1, 0:1])
            nc.gpsimd.dma_start(ccx_in[1:2], redgw[0:1, 1:2])
            nc.gpsimd.collective_compute(
                "AllGather", Alu.bypass,
                replica_groups=[list(range(n_cores))],
                ins=[ccx_in.ap()], outs=[ccx_out.ap()])
            nc.gpsimd.dma_start(
                scx1, ccx_out.ap().rearrange("(a b) -> a b", a=1))
            nc.gpsimd.partition_broadcast(scx, scx1)
            scx_v = scx.rearrange("p (c s) -> p s c", s=2)

            # gamma = max over cores; s16 = 16/gamma
            nc.vector.tensor_reduce(scal[:, 0:1], scx_v[:, 0, :],
                                    axis=mybir.AxisListType.X, op=Alu.max)
            nc.vector.tensor_scalar_max(scal[:, 0:1], scal[:, 0:1], EPS)
            nc.vector.reciprocal(n16, scal[:, 0:1])
            nc.vector.tensor_scalar_mul(scal[:, 3:4], n16, 16.0)

            # beta = sum over cores / n_total; thr = beta/2;
            # c_out = beta*gamma/16
            nc.vector.tensor_reduce(scal[:, 1:2], scx_v[:, 1, :],
                                    axis=mybir.AxisListType.X, op=Alu.add)
            inv_n = float(np.float32(1.0) / np.float32(n_total))
            nc.vector.tensor_scalar_mul(scal[:, 2:3], scal[:, 1:2], inv_n)
            nc.vector.tensor_scalar_max(scal[:, 2:3], scal[:, 2:3], EPS)
            nc.vector.tensor_scalar_mul(scal[:, 4:5], scal[:, 2:3], 0.5)
            nc.vector.tensor_tensor(scal[:, 6:7], scal[:, 4:5],
                                    scal[:, 4:5], op=Alu.mult)
            nc.vector.tensor_tensor(scal[:, 5:6], scal[:, 2:3],
                                    scal[:, 0:1], op=Alu.mult)
            nc.vector.tensor_scalar_mul(scal[:, 5:6], scal[:, 5:6],
                                        1.0 / 16.0)

            ab_tiles = {}

            def quantize_block(b, on_act):
                ab8 = ab_pool.tile([128, KT, 2, blk], fp8, name="ab8")
                ab_tiles[b] = ab8
                for qq in range(NQ):
                    x_t = x_pool.tile([128, KQ, blk], f32, tag="x_t",
                                      name="x_t")
                    nc.gpsimd.dma_start(
                        x_t, xT_r[:, ts(qq, KQ), ts(b, blk)])
                    a_sl = ab8[:, ts(qq, KQ), 0, :]
                    if on_act:
                        nc.scalar.activation(x_t, x_t, Act.Copy,
                                             scale=scal[:, 3:4])
                        nc.scalar.activation(a_sl, x_t, Act.Copy)
                    else:
                        nc.vector.tensor_scalar_mul(x_t, x_t,
                                                    scal[:, 3:4])
                        nc.vector.tensor_scalar_mul(a_sl, x_t, 1.0)
                    nc.vector.tensor_tensor(
                        ab8[:, ts(qq, KQ), 1, :], x_t, a_sl,
                        op=Alu.subtract)

            # block 0 quantizes on DVE, ahead of the W stream in DVE
            # program order (gated on gamma only -> PE trails the w8
            # stream through tt0); later blocks go on ACT, pre-emitted
            # a block ahead.
            quantize_block(0, on_act=False)

            # ---- W quantize stream ----
            for k in range(KT):
                wt_t = wt_pool.tile([128, O_SH], f32, tag="wtile",
                                    name="wq_t")
                nc.sync.dma_start(wt_t, wt_r[:, k, :])
                nc.vector.tensor_tensor(wt_t, wt_t, wt_t, op=Alu.mult)
                nc.vector.tensor_scalar(w8[:, k, :], wt_t, scal[:, 6:7],
                                        None, op0=Alu.is_gt)

            def ps_sections(sec_list):
                for (soff, sw) in sec_list:
                    psS = pse_pool.tile([128, 512], f32,
                                        tag="pset", name="psS")
                    for kk in range(KK):
                        nc.tensor.matmul(
                            psS[0:1, 0:sw], ones_lhs,
                            w8[:, 2 * kk:2 * kk + 2, soff:soff + sw],
                            start=(kk == 0), stop=(kk == KK - 1),
                            perf_mode=DR)
                    nc.vector.tensor_scalar_mul(
                        wx8[0:1, 0, soff:soff + sw],
                        psS[0:1, 0:sw], inv_I)

            # ---- main loop ----
            for b in range(NBLK):
                aext = aextA if b % 2 == 0 else aextB
                if b not in ab_tiles:
                    quantize_block(b, on_act=True)
                ab8 = ab_tiles[b]

                for tt in range(TPB):
                    if tt == 1 and 1 <= b + 1 < NBLK \
                            and b + 1 not in ab_tiles:
                        # pre-quantize next block here: its ACT/DVE ops
                        # land between tt0's and tt1's copies in those
                        # FIFOs, hiding under tt1's matmuls
                        quantize_block(b + 1, on_act=True)
                    first = (b == 0 and tt == 0)
                    ph = [ps_pool.tile([128, OH], f32, tag=f"ph{h}",
                                       name=f"ph{h}") for h in range(2)]
                    for kk in range(KK):
                        lhsT = ab8[:, 2 * kk:2 * kk + 2, 0, ts(tt, 128)]
                        for h in range(2):
                            base = h * OH
                            for (off, w_, rs, re) in hchunks:
                                nc.tensor.matmul(
                                    ph[h][:, off:off + w_], lhsT,
                                    w8[:, 2 * kk:2 * kk + 2,
                                       base + off:base + off + w_],
                                    start=(kk == 0 and rs), stop=False,
                                    perf_mode=DR)
                    if tt == 0:
                        # E[t] = sum_k B[t,k] -> aext fp8 row
                        psE = pse_pool.tile([128, 512], f32, tag="pset",
                                            name="psE")
                        for kk in range(KK):
                            nc.tensor.matmul(
                                psE[0:1, 0:blk], ones_lhs,
                                ab8[:, 2 * kk:2 * kk + 2, 1, :],
                                start=(kk == 0), stop=(kk == KK - 1),
                                perf_mode=DR)
                        nc.vector.tensor_scalar_mul(aext[0:1, 0, :],
                                                    psE[0:1, 0:blk], 1.0)

                    for h in range(2):
                        if first:
                            ps_sections(secs_h0 if h == 0 else secs_h1)
                        base = h * OH
                        for (off, w_, rs, re) in hchunks:
                            nc.tensor.matmul(
                                ph[h][:, off:off + w_],
                                aext[:, :, ts(tt, 128)],
                                wx8[:, :, base + off:base + off + w_],
                                start=False, stop=re, perf_mode=DR)
                        y_t = y_pool.tile([128, OH], bf16, tag="yh",
                                          name="y_t")
                        if h == 0:
                            nc.scalar.activation(y_t, ph[h], Act.Copy,
                                                 scale=scal[:, 5:6])
                        else:
                            nc.vector.tensor_scalar(y_t, ph[h],
                                                    scal[:, 5:6], None,
                                                    op0=Alu.mult)
                        nc.gpsimd.dma_start(
                            y_d.ap()[ts(b * TPB + tt, 128),
                                     base:base + OH], y_t)

    nc.compile()
    _dedup_ldweights(nc)
    return nc


def _dedup_ldweights(nc):
    """Drop InstLdweights whose weights AP equals the previous PE load."""
    removed = kept_sync = 0
    for fn in nc.m.functions:
        for blk_ in fn.blocks:
            insts = blk_.instructions
            prev_sig = None
            kill = []
            for j, ins in enumerate(insts):
                tn = type(ins).__name__
                if tn == "InstLdweights":
                    sig = (str(ins.ins[0]), str(ins.perf_mode),
                           str(ins.is_transpose))
                    if sig == prev_sig:
                        if not ins.has_wait() and not ins.has_update():
                            kill.append(j)
                        else:
                            kept_sync += 1
                    prev_sig = sig
                elif tn == "InstMatmult":
                    if ins.is_transpose:
                        prev_sig = None
            for j in reversed(kill):
                del insts[j]
            removed += len(kill)
    if removed:
        print(f"[kernel_sp] deduped {removed} redundant ldweights "
              f"({kept_sync} kept for sync)")


_CACHED_NC = None


def _get_nc():
    global _CACHED_NC
    if _CACHED_NC is None:
        _CACHED_NC = build_kernel()
    return _CACHED_NC


def shard_inputs(x, weight):
    x2 = x.reshape(TOK_TOTAL, I_DIM).astype(np.float32, copy=False)
    weight = weight.astype(np.float32, copy=False)
    xT_halves = [
        np.ascontiguousarray(x2[h * TOK:(h + 1) * TOK].T)
        for h in range(TOK_HALVES)
    ]
    wt_quarters = [
        np.ascontiguousarray(weight[q * O_SH:(q + 1) * O_SH].T)
        for q in range(O_QUARTERS)
    ]
    gk = I_DIM // O_QUARTERS
    bk = I_DIM // TOK_HALVES
    in_maps = []
    for c in range(N_CORES):
        h, q = c // O_QUARTERS, c % O_QUARTERS
        in_maps.append({
            "xT": xT_halves[h],
            "wt": wt_quarters[q],
            "xg": np.ascontiguousarray(xT_halves[h][q * gk:(q + 1) * gk]),
            "wb": np.ascontiguousarray(wt_quarters[q][h * bk:(h + 1) * bk]),
        })
    return in_maps


def unshard_output(results):
    rows = []
    for h in range(TOK_HALVES):
        cols = [np.asarray(results[h * O_QUARTERS + q]["y"])
                for q in range(O_QUARTERS)]
        rows.append(np.concatenate(cols, axis=1))
    y = np.concatenate(rows, axis=0).astype(np.float32)
    return y.reshape(B_DIM, S_DIM, O_DIM)


def run_on_cores(x, weight, trace=False):
    from concourse.bass_utils import run_bass_kernel_spmd
    nc = _get_nc()
    in_maps = shard_inputs(x, weight)
    res = run_bass_kernel_spmd(nc, in_maps, core_ids=list(range(N_CORES)),
                               trace=trace)
    return res


def kernel(x, weight):
    res = run_on_cores(x, weight, trace=False)
    return unshard_output(res.results)


# revision 23
# speedup vs baseline: 1.0109x; 1.0057x over previous
"""BitLinear (B=8) TRN2 kernel — single-pass fp8 DoubleRow + rank-1 correction.

Math (reference):
    gamma = max(max|x|, 1e-5);  xq = clip(round(x*256/gamma), -256, 255)
    beta  = max(mean|W|, 1e-5); wq = (|W| > 0.5*beta)  in {0,1}
    y     = (xq @ wq.T) * (beta*gamma/256)

Scheme: u = x*(16/gamma) in [-16,16];  a = e4m3(u)  (one fp8 DoubleRow
pass, 2 k-tiles per instruction);  residual e = u - a is corrected by the
rank-1 term  (sum_k e[t,k]) * (colsum(wq)[o] / I), folded into the matmul
as one extra DoubleRow contraction step whose stationary row is
E8[t] = fp8(sum_k B[t,k]) (partition 0 only) and whose moving row is
s8[o] = fp8(colsum(wq)[o]/4096).  Measured rel err ~1.5e-2 (gate 2e-2).

Pipeline design (engine-FIFO aware):
  - wb (beta partial) streams in 2-ktile DMAs; gamma+beta partials ride
    ONE fused AllGather.
  - wt stream: DMA (sync q) -> abs (ACT) -> is_gt (DVE).  The DMA+abs
    run bufs-ahead from t=0; only is_gt waits on beta.
  - blocks 0-1 of x are quantized on DVE *before* the W stream in DVE
    program order (gated on gamma only), so tt0's accumulation matmuls
    trail the w8 stream as k-tiles arrive; blocks 2+ quantize on ACT
    (emitted after the paced wt-abs ops, by which time the stream has
    drained).
  - psum is split into two half-tiles (3 banks each); each half's
    rank-1 + copy happens independently so the copy of one half hides
    under the other half's matmuls (no per-tt PE bubble).
  - psS (colsum of wq) is sectioned through the pse psum slot inside
    tt0, after the kk-loop, with each half's rank-1 gated only on the
    sections it reads.

Distribution: 2x4 grid (token halves x out-feature quarters), x shipped
host-transposed so the contraction lands on partitions with no on-device
transpose; gamma/beta via per-core disjoint partials + one AllGather.

A post-compile pass drops InstLdweights whose weights AP equals the
previous load on the PE stream.
"""

import numpy as np

# ---- problem constants (hardcoded; kernel.py must be self-contained) ----
B_DIM, S_DIM, I_DIM, O_DIM = 4, 2048, 4096, 11008
N_CORES = 8
TOK_HALVES = 2
O_QUARTERS = 4
TOK_TOTAL = B_DIM * S_DIM
TOK = TOK_TOTAL // TOK_HALVES       # 4096 tokens per core
O_SH = O_DIM // O_QUARTERS          # 2752 out features per core

EPS = 1e-5


def _half_chunks(width):
    """256-wide chunks that never cross a 512-f32 psum region boundary;
    yields (off, w, region_start, region_end)."""
    chunks = []
    off = 0
    while off < width:
        rem = width - off
        w = min(256, rem)
        # absorb a short tail into one wider chunk if it stays in-region
        if rem <= 512 - (off % 512):
            w = rem
        rs = off % 512 == 0
        re = (off + w) % 512 == 0 or off + w == width
        chunks.append((off, w, rs, re))
        off += w
    return chunks


def build_kernel(I=I_DIM, TOK=TOK, O_SH=O_SH, n_cores=N_CORES,
                 tok_halves=TOK_HALVES, o_quarters=O_QUARTERS,
                 n_total=None, blk=256):
    """Per-core: xT [I, TOK] f32, wt [I, O_SH] f32, xg/wb partial slices.
    Output: y [TOK, O_SH] bf16."""
    import concourse.bacc as bacc
    import concourse.mybir as mybir
    import concourse.tile as tile
    from concourse import bass_isa
    from concourse.bass import ts

    if n_total is None:
        n_total = float(I) * float(O_SH * o_quarters)

    f32 = mybir.dt.float32
    bf16 = mybir.dt.bfloat16
    fp8 = mybir.dt.float8e4
    Alu = mybir.AluOpType
    Act = mybir.ActivationFunctionType
    DR = mybir.MatmulPerfMode.DoubleRow

    KT = I // 128
    KK = KT // 2                    # DoubleRow k-pair steps
    KQ = min(8, KT)                 # k-tiles per quantize step
    NQ = KT // KQ
    NBLK = TOK // blk
    TPB = blk // 128
    GKT = (I // o_quarters) // 128
    WKT = (I // tok_halves) // 128
    WB2 = 1                         # k-tiles per beta DMA
    inv_I = float(np.float32(1.0) / np.float32(I))

    OH = O_SH // 2                  # evacuation half width
    hchunks = _half_chunks(OH)      # within-half (off, w, rs, re)
    # psS sections (512-wide over full O_SH); h0's rank-1 needs only the
    # sections overlapping [0, OH)
    secs = []
    off = 0
    while off < O_SH:
        sw = min(512, O_SH - off)
        secs.append((off, sw))
        off += sw
    secs_h0 = [s for s in secs if s[0] < OH]
    secs_h1 = [s for s in secs if s[0] >= OH]

    nc = bacc.Bacc("TRN2", target_bir_lowering=False, debug=False,
                   num_devices=n_cores)

    xT_d = nc.dram_tensor("xT", [I, TOK], f32, kind="ExternalInput")
    wt_d = nc.dram_tensor("wt", [I, O_SH], f32, kind="ExternalInput")
    xg_d = nc.dram_tensor("xg", [I // o_quarters, TOK], f32,
                          kind="ExternalInput")
    wb_d = nc.dram_tensor("wb", [I // tok_halves, O_SH], f32,
                          kind="ExternalInput")
    y_d = nc.dram_tensor("y", [TOK, O_SH], bf16, kind="ExternalOutput")
    shared = "Shared" if n_cores > 4 else "Local"
    ccx_in = nc.dram_tensor("ccx_in", [2], f32)
    ccx_out = nc.dram_tensor("ccx_out", [2 * n_cores], f32,
                             addr_space=shared)

    xT_r = xT_d.ap().rearrange("(kt p) m -> p kt m", p=128)
    wt_r = wt_d.ap().rearrange("(kt p) o -> p kt o", p=128)
    xg_r = xg_d.ap().rearrange("(kt p) m -> p kt m", p=128)
    wb_r = wb_d.ap().rearrange("(kt p) o -> p kt o", p=128)

    with tile.TileContext(nc) as tc:
        with (
            tc.tile_pool(name="wtp", bufs=8) as wt_pool,
            tc.tile_pool(name="wbp", bufs=2) as wb_pool,
            tc.tile_pool(name="xs", bufs=2) as x_pool,
            tc.tile_pool(name="ab", bufs=2) as ab_pool,
            tc.tile_pool(name="wres", bufs=1) as wres_pool,
            tc.tile_pool(name="stat", bufs=1) as stat_pool,
            tc.tile_pool(name="yout", bufs=1) as y_pool,
            tc.tile_pool(name="ps", bufs=1, space="PSUM") as ps_pool,
            tc.tile_pool(name="pse", bufs=2, space="PSUM") as pse_pool,
        ):
            w8 = wres_pool.tile([128, KT, O_SH], fp8)       # wq in {0,1}
            wx8 = wres_pool.tile([128, 2, O_SH], fp8)       # ext weights row
            ones8 = stat_pool.tile([128, 2, 16], fp8)
            gchunk = min(TOK, 2048)
            gsub = TOK // gchunk
            gmax1 = stat_pool.tile([1, GKT * gsub], f32)
            redg1 = stat_pool.tile([1, 1], f32)
            wsum = stat_pool.tile([128, WKT // WB2], f32)
            redgw = stat_pool.tile([128, 2], f32)           # [gamma, beta]
            scx1 = stat_pool.tile([1, 2 * n_cores], f32)
            scx = stat_pool.tile([128, 2 * n_cores], f32)
            scal = stat_pool.tile([128, 8], f32)
            n16 = stat_pool.tile([128, 1], f32)
            aextA = stat_pool.tile([128, 2, blk], fp8)      # ext activations
            aextB = aextA
            redw = stat_pool.tile([128, 1], f32)

            nc.vector.memset(wx8.rearrange("p a o -> p (a o)"), 0.0)
            nc.vector.memset(aextA.rearrange("p a m -> p (a m)"), 0.0)
            nc.vector.memset(ones8.rearrange("p a b -> p (a b)"), 1.0)
            ones_lhs = ones8[:, :, 0:1]

            # ---- gamma partial: max|xg| (DVE; xg on scalar queue) ----
            for i in range(GKT):
                for j in range(gsub):
                    gx_t = x_pool.tile([128, gchunk], f32, tag="x_t",
                                       name="gx_t")
                    nc.gpsimd.dma_start(gx_t, xg_r[:, i, ts(j, gchunk)])
                    nc.gpsimd.tensor_reduce(
                        gmax1[0:1, i * gsub + j:i * gsub + j + 1], gx_t,
                        axis=mybir.AxisListType.XYZWC,
                        op=Alu.max, apply_absolute_value=True)
            nc.gpsimd.tensor_reduce(redg1, gmax1,
                                    axis=mybir.AxisListType.XYZWC,
                                    op=Alu.max)

            # ---- beta partial: sum|wb| in 2-ktile strides (ACT abs) ----
            for i in range(WKT // WB2):
                wb_t = wb_pool.tile([128, WB2, O_SH], f32, tag="wbtile",
                                    name="wb_t")
                nc.sync.dma_start(wb_t, wb_r[:, ts(i, WB2), :])
                nc.vector.tensor_reduce(
                    wsum[:, i:i + 1],
                    wb_t.rearrange("p a o -> p (a o)"),
                    axis=mybir.AxisListType.X, op=Alu.add,
                    apply_absolute_value=True)
            nc.vector.tensor_reduce(redw, wsum,
                                    axis=mybir.AxisListType.X, op=Alu.add)

            # ---- fused cross-partition + cross-core reduction ----
            nc.gpsimd.partition_all_reduce(redgw[:, 1:2], redw, channels=128,
                                           reduce_op=bass_isa.ReduceOp.add)
            nc.gpsimd.dma_start(ccx_in[0:1], redg1[0:1, 0:1])
            nc.gpsimd.dma_start(ccx_in[1:2], redgw[0:1, 1:2])
            nc.gpsimd.collective_compute(
                "AllGather", Alu.bypass,
                replica_groups=[list(range(n_cores))],
                ins=[ccx_in.ap()], outs=[ccx_out.ap()])
            nc.gpsimd.dma_start(
                scx1, ccx_out.ap().rearrange("(a b) -> a b", a=1))
            nc.gpsimd.partition_broadcast(scx, scx1)
            scx_v = scx.rearrange("p (c s) -> p s c", s=2)

            # gamma = max over cores; s16 = 16/gamma
            nc.vector.tensor_reduce(scal[:, 0:1], scx_v[:, 0, :],
                                    axis=mybir.AxisListType.X, op=Alu.max)
            nc.vector.tensor_scalar_max(scal[:, 0:1], scal[:, 0:1], EPS)
            nc.vector.reciprocal(n16, scal[:, 0:1])
            nc.vector.tensor_scalar_mul(scal[:, 3:4], n16, 16.0)

            # beta = sum over cores / n_total; thr = beta/2;
            # c_out = beta*gamma/16
            nc.vector.tensor_reduce(scal[:, 1:2], scx_v[:, 1, :],
                                    axis=mybir.AxisListType.X, op=Alu.add)
            inv_n = float(np.float32(1.0) / np.float32(n_total))
            nc.vector.tensor_scalar_mul(scal[:, 2:3], scal[:, 1:2], inv_n)
            nc.vector.tensor_scalar_max(scal[:, 2:3], scal[:, 2:3], EPS)
            nc.vector.tensor_scalar_mul(scal[:, 4:5], scal[:, 2:3], 0.5)
            nc.vector.tensor_tensor(scal[:, 6:7], scal[:, 4:5],
                                    scal[:, 4:5], op=Alu.mult)
            nc.vector.tensor_tensor(scal[:, 5:6], scal[:, 2:3],
                                    scal[:, 0:1], op=Alu.mult)
            nc.vector.tensor_scalar_mul(scal[:, 5:6], scal[:, 5:6],
                                        1.0 / 16.0)

            ab_tiles = {}

            def quantize_block(b, on_act):
                ab8 = ab_pool.tile([128, KT, 2, blk], fp8, name="ab8")
                ab_tiles[b] = ab8
                for qq in range(NQ):
                    x_t = x_pool.tile([128, KQ, blk], f32, tag="x_t",
                                      name="x_t")
                    nc.gpsimd.dma_start(
                        x_t, xT_r[:, ts(qq, KQ), ts(b, blk)])
                    a_sl = ab8[:, ts(qq, KQ), 0, :]
                    if on_act:
                        nc.scalar.activation(x_t, x_t, Act.Copy,
                                             scale=scal[:, 3:4])
                        nc.scalar.activation(a_sl, x_t, Act.Copy)
                    else:
                        nc.vector.tensor_scalar_mul(x_t, x_t,
                                                    scal[:, 3:4])
                        nc.vector.tensor_scalar_mul(a_sl, x_t, 1.0)
                    nc.vector.tensor_tensor(
                        ab8[:, ts(qq, KQ), 1, :], x_t, a_sl,
                        op=Alu.subtract)

            # block 0 quantizes on DVE, ahead of the W stream in DVE
            # program order (gated on gamma only -> PE trails the w8
            # stream through tt0); later blocks go on ACT, pre-emitted
            # a block ahead.
            quantize_block(0, on_act=False)

            # ---- W quantize stream: half-width tiles, 8 bufs (same
            # SBUF as 4 full tiles) -> deeper pipeline absorbs the
            # per-hop latency jitter; pace stays DMA-bound
            WH = O_SH // 2
            for k in range(KT):
                for h in range(2):
                    wt_t = wt_pool.tile([128, WH], f32, tag="wtile",
                                        name="wq_t")
                    nc.sync.dma_start(
                        wt_t, wt_r[:, k, h * WH:(h + 1) * WH])
                    nc.vector.tensor_tensor(wt_t, wt_t, wt_t,
                                            op=Alu.mult)
                    nc.vector.tensor_scalar(
                        w8[:, k, h * WH:(h + 1) * WH], wt_t,
                        scal[:, 6:7], None, op0=Alu.is_gt)

            def ps_sections(sec_list):
                for (soff, sw) in sec_list:
                    psS = pse_pool.tile([128, 512], f32,
                                        tag="pset", name="psS")
                    for kk in range(KK):
                        nc.tensor.matmul(
                            psS[0:1, 0:sw], ones_lhs,
                            w8[:, 2 * kk:2 * kk + 2, soff:soff + sw],
                            start=(kk == 0), stop=(kk == KK - 1),
                            perf_mode=DR)
                    nc.vector.tensor_scalar_mul(
                        wx8[0:1, 0, soff:soff + sw],
                        psS[0:1, 0:sw], inv_I)

            # ---- main loop ----
            for b in range(NBLK):
                aext = aextA if b % 2 == 0 else aextB
                if b not in ab_tiles:
                    quantize_block(b, on_act=True)
                ab8 = ab_tiles[b]

                for tt in range(TPB):
                    if tt == 1 and 1 <= b + 1 < NBLK \
                            and b + 1 not in ab_tiles:
                        # pre-quantize next block here: its ACT/DVE ops
                        # land between tt0's and tt1's copies in those
                        # FIFOs, hiding under tt1's matmuls
                        quantize_block(b + 1, on_act=True)
                    first = (b == 0 and tt == 0)
                    ph = [ps_pool.tile([128, OH], f32, tag=f"ph{h}",
                                       name=f"ph{h}") for h in range(2)]
                    for kk in range(KK):
                        lhsT = ab8[:, 2 * kk:2 * kk + 2, 0, ts(tt, 128)]
                        for h in range(2):
                            base = h * OH
                            for (off, w_, rs, re) in hchunks:
                                nc.tensor.matmul(
                                    ph[h][:, off:off + w_], lhsT,
                                    w8[:, 2 * kk:2 * kk + 2,
                                       base + off:base + off + w_],
                                    start=(kk == 0 and rs), stop=False,
                                    perf_mode=DR)
                    if tt == 0:
                        # E[t] = sum_k B[t,k] -> aext fp8 row
                        psE = pse_pool.tile([128, 512], f32, tag="pset",
                                            name="psE")
                        for kk in range(KK):
                            nc.tensor.matmul(
                                psE[0:1, 0:blk], ones_lhs,
                                ab8[:, 2 * kk:2 * kk + 2, 1, :],
                                start=(kk == 0), stop=(kk == KK - 1),
                                perf_mode=DR)
                        nc.vector.tensor_scalar_mul(aext[0:1, 0, :],
                                                    psE[0:1, 0:blk], 1.0)

                    for h in range(2):
                        if first:
                            ps_sections(secs_h0 if h == 0 else secs_h1)
                        base = h * OH
                        for (off, w_, rs, re) in hchunks:
                            nc.tensor.matmul(
                                ph[h][:, off:off + w_],
                                aext[:, :, ts(tt, 128)],
                                wx8[:, :, base + off:base + off + w_],
                                start=False, stop=re, perf_mode=DR)
                        y_t = y_pool.tile([128, OH], bf16, tag="yh",
                                          name="y_t")
                        if h == 0:
                            nc.scalar.activation(y_t, ph[h], Act.Copy,
                                                 scale=scal[:, 5:6])
                        else:
                            nc.vector.tensor_scalar(y_t, ph[h],
                                                    scal[:, 5:6], None,
                                                    op0=Alu.mult)
                        nc.gpsimd.dma_start(
                            y_d.ap()[ts(b * TPB + tt, 128),
                                     base:base + OH], y_t)

    nc.compile()
    _dedup_ldweights(nc)
    return nc


def _dedup_ldweights(nc):
    """Drop InstLdweights whose weights AP equals the previous PE load."""
    removed = kept_sync = 0
    for fn in nc.m.functions:
        for blk_ in fn.blocks:
            insts = blk_.instructions
            prev_sig = None
            kill = []
            for j, ins in enumerate(insts):
                tn = type(ins).__name__
                if tn == "InstLdweights":
                    sig = (str(ins.ins[0]), str(ins.perf_mode),
                           str(ins.is_transpose))
                    if sig == prev_sig:
                        if not ins.has_wait() and not ins.has_update():
                            kill.append(j)
                        else:
                            kept_sync += 1
                    prev_sig = sig
                elif tn == "InstMatmult":
                    if ins.is_transpose:
                        prev_sig = None
            for j in reversed(kill):
                del insts[j]
            removed += len(kill)
    if removed:
        print(f"[kernel_sp] deduped {removed} redundant ldweights "
              f"({kept_sync} kept for sync)")


_CACHED_NC = None


def _get_nc():
    global _CACHED_NC
    if _CACHED_NC is None:
        _CACHED_NC = build_kernel()
    return _CACHED_NC


def shard_inputs(x, weight):
    x2 = x.reshape(TOK_TOTAL, I_DIM).astype(np.float32, copy=False)
    weight = weight.astype(np.float32, copy=False)
    xT_halves = [
        np.ascontiguousarray(x2[h * TOK:(h + 1) * TOK].T)
        for h in range(TOK_HALVES)
    ]
    wt_quarters = [
        np.ascontiguousarray(weight[q * O_SH:(q + 1) * O_SH].T)
        for q in range(O_QUARTERS)
    ]
    gk = I_DIM // O_QUARTERS
    bk = I_DIM // TOK_HALVES
    in_maps = []
    for c in range(N_CORES):
        h, q = c // O_QUARTERS, c % O_QUARTERS
        in_maps.append({
            "xT": xT_halves[h],
            "wt": wt_quarters[q],
            "xg": np.ascontiguousarray(xT_halves[h][q * gk:(q + 1) * gk]),
            "wb": np.ascontiguousarray(wt_quarters[q][h * bk:(h + 1) * bk]),
        })
    return in_maps


def unshard_output(results):
    rows = []
    for h in range(TOK_HALVES):
        cols = [np.asarray(results[h * O_QUARTERS + q]["y"])
                for q in range(O_QUARTERS)]
        rows.append(np.concatenate(cols, axis=1))
    y = np.concatenate(rows, axis=0).astype(np.float32)
    return y.reshape(B_DIM, S_DIM, O_DIM)


def run_on_cores(x, weight, trace=False):
    from concourse.bass_utils import run_bass_kernel_spmd
    nc = _get_nc()
    in_maps = shard_inputs(x, weight)
    res = run_bass_kernel_spmd(nc, in_maps, core_ids=list(range(N_CORES)),
                               trace=trace)
    return res


def kernel(x, weight):
    res = run_on_cores(x, weight, trace=False)
    return unshard_output(res.results)


# revision 24
# speedup vs baseline: 1.0112x; 1.0003x over previous
"""BitLinear (B=8) TRN2 kernel — single-pass fp8 DoubleRow + rank-1 correction.

Math (reference):
    gamma = max(max|x|, 1e-5);  xq = clip(round(x*256/gamma), -256, 255)
    beta  = max(mean|W|, 1e-5); wq = (|W| > 0.5*beta)  in {0,1}
    y     = (xq @ wq.T) * (beta*gamma/256)

Scheme: u = x*(16/gamma) in [-16,16];  a = e4m3(u)  (one fp8 DoubleRow
pass, 2 k-tiles per instruction);  residual e = u - a is corrected by the
rank-1 term  (sum_k e[t,k]) * (colsum(wq)[o] / I), folded into the matmul
as one extra DoubleRow contraction step whose stationary row is
E8[t] = fp8(sum_k B[t,k]) (partition 0 only) and whose moving row is
s8[o] = fp8(colsum(wq)[o]/4096).  Measured rel err ~1.5e-2 (gate 2e-2).

Pipeline design (engine-FIFO aware):
  - wb (beta partial) streams in 2-ktile DMAs; gamma+beta partials ride
    ONE fused AllGather.
  - wt stream: DMA (sync q) -> abs (ACT) -> is_gt (DVE).  The DMA+abs
    run bufs-ahead from t=0; only is_gt waits on beta.
  - blocks 0-1 of x are quantized on DVE *before* the W stream in DVE
    program order (gated on gamma only), so tt0's accumulation matmuls
    trail the w8 stream as k-tiles arrive; blocks 2+ quantize on ACT
    (emitted after the paced wt-abs ops, by which time the stream has
    drained).
  - psum is split into two half-tiles (3 banks each); each half's
    rank-1 + copy happens independently so the copy of one half hides
    under the other half's matmuls (no per-tt PE bubble).
  - psS (colsum of wq) is sectioned through the pse psum slot inside
    tt0, after the kk-loop, with each half's rank-1 gated only on the
    sections it reads.

Distribution: 2x4 grid (token halves x out-feature quarters), x shipped
host-transposed so the contraction lands on partitions with no on-device
transpose; gamma/beta via per-core disjoint partials + one AllGather.

A post-compile pass drops InstLdweights whose weights AP equals the
previous load on the PE stream.
"""

import numpy as np

# ---- problem constants (hardcoded; kernel.py must be self-contained) ----
B_DIM, S_DIM, I_DIM, O_DIM = 4, 2048, 4096, 11008
N_CORES = 8
TOK_HALVES = 2
O_QUARTERS = 4
TOK_TOTAL = B_DIM * S_DIM
TOK = TOK_TOTAL // TOK_HALVES       # 4096 tokens per core
O_SH = O_DIM // O_QUARTERS          # 2752 out features per core

EPS = 1e-5


def _half_chunks(width):
    """256-wide chunks that never cross a 512-f32 psum region boundary;
    yields (off, w, region_start, region_end)."""
    chunks = []
    off = 0
    while off < width:
        rem = width - off
        w = min(256, rem)
        # absorb a short tail into one wider chunk if it stays in-region
        if rem <= 512 - (off % 512):
            w = rem
        rs = off % 512 == 0
        re = (off + w) % 512 == 0 or off + w == width
        chunks.append((off, w, rs, re))
        off += w
    return chunks


def build_kernel(I=I_DIM, TOK=TOK, O_SH=O_SH, n_cores=N_CORES,
                 tok_halves=TOK_HALVES, o_quarters=O_QUARTERS,
                 n_total=None, blk=256):
    """Per-core: xT [I, TOK] f32, wt [I, O_SH] f32, xg/wb partial slices.
    Output: y [TOK, O_SH] bf16."""
    import concourse.bacc as bacc
    import concourse.mybir as mybir
    import concourse.tile as tile
    from concourse import bass_isa
    from concourse.bass import ts

    if n_total is None:
        n_total = float(I) * float(O_SH * o_quarters)

    f32 = mybir.dt.float32
    bf16 = mybir.dt.bfloat16
    fp8 = mybir.dt.float8e4
    Alu = mybir.AluOpType
    Act = mybir.ActivationFunctionType
    DR = mybir.MatmulPerfMode.DoubleRow

    KT = I // 128
    KK = KT // 2                    # DoubleRow k-pair steps
    KQ = min(8, KT)                 # k-tiles per quantize step
    NQ = KT // KQ
    NBLK = TOK // blk
    TPB = blk // 128
    GKT = (I // o_quarters) // 128
    WKT = (I // tok_halves) // 128
    WB2 = 1                         # k-tiles per beta DMA
    inv_I = float(np.float32(1.0) / np.float32(I))

    OH = O_SH // 2                  # evacuation half width
    hchunks = _half_chunks(OH)      # within-half (off, w, rs, re)
    # psS sections (512-wide over full O_SH); h0's rank-1 needs only the
    # sections overlapping [0, OH)
    secs = []
    off = 0
    while off < O_SH:
        sw = min(512, O_SH - off)
        secs.append((off, sw))
        off += sw
    secs_h0 = [s for s in secs if s[0] < OH]
    secs_h1 = [s for s in secs if s[0] >= OH]

    nc = bacc.Bacc("TRN2", target_bir_lowering=False, debug=False,
                   num_devices=n_cores)

    xT_d = nc.dram_tensor("xT", [I, TOK], f32, kind="ExternalInput")
    wt_d = nc.dram_tensor("wt", [I, O_SH], f32, kind="ExternalInput")
    xg_d = nc.dram_tensor("xg", [I // o_quarters, TOK], f32,
                          kind="ExternalInput")
    wb_d = nc.dram_tensor("wb", [I // tok_halves, O_SH], f32,
                          kind="ExternalInput")
    y_d = nc.dram_tensor("y", [TOK, O_SH], bf16, kind="ExternalOutput")
    shared = "Shared" if n_cores > 4 else "Local"
    ccx_in = nc.dram_tensor("ccx_in", [2], f32)
    ccx_out = nc.dram_tensor("ccx_out", [2 * n_cores], f32,
                             addr_space=shared)

    xT_r = xT_d.ap().rearrange("(kt p) m -> p kt m", p=128)
    wt_r = wt_d.ap().rearrange("(kt p) o -> p kt o", p=128)
    xg_r = xg_d.ap().rearrange("(kt p) m -> p kt m", p=128)
    wb_r = wb_d.ap().rearrange("(kt p) o -> p kt o", p=128)

    with tile.TileContext(nc) as tc:
        with (
            tc.tile_pool(name="wtp", bufs=4) as wt_pool,
            tc.tile_pool(name="wbp", bufs=2) as wb_pool,
            tc.tile_pool(name="xs", bufs=2) as x_pool,
            tc.tile_pool(name="ab", bufs=2) as ab_pool,
            tc.tile_pool(name="wres", bufs=1) as wres_pool,
            tc.tile_pool(name="stat", bufs=1) as stat_pool,
            tc.tile_pool(name="yout", bufs=1) as y_pool,
            tc.tile_pool(name="ps", bufs=1, space="PSUM") as ps_pool,
            tc.tile_pool(name="pse", bufs=2, space="PSUM") as pse_pool,
        ):
            w8 = wres_pool.tile([128, KT, O_SH], fp8)       # wq in {0,1}
            wx8 = wres_pool.tile([128, 2, O_SH], fp8)       # ext weights row
            ones8 = stat_pool.tile([128, 2, 16], fp8)
            gchunk = min(TOK, 2048)
            gsub = TOK // gchunk
            gmax1 = stat_pool.tile([1, GKT * gsub], f32)
            redg1 = stat_pool.tile([1, 1], f32)
            wsum = stat_pool.tile([128, WKT // WB2], f32)
            redgw = stat_pool.tile([128, 2], f32)           # [gamma, beta]
            scx1 = stat_pool.tile([1, 2 * n_cores], f32)
            scx = stat_pool.tile([128, 2 * n_cores], f32)
            scal = stat_pool.tile([128, 8], f32)
            n16 = stat_pool.tile([128, 1], f32)
            aextA = stat_pool.tile([128, 2, blk], fp8)      # ext activations
            aextB = aextA
            redw = stat_pool.tile([128, 1], f32)

            nc.vector.memset(wx8.rearrange("p a o -> p (a o)"), 0.0)
            nc.vector.memset(aextA.rearrange("p a m -> p (a m)"), 0.0)
            nc.vector.memset(ones8.rearrange("p a b -> p (a b)"), 1.0)
            ones_lhs = ones8[:, :, 0:1]

            # ---- gamma partial: max|xg| (DVE; xg on scalar queue) ----
            for i in range(GKT):
                for j in range(gsub):
                    gx_t = x_pool.tile([128, gchunk], f32, tag="x_t",
                                       name="gx_t")
                    nc.gpsimd.dma_start(gx_t, xg_r[:, i, ts(j, gchunk)])
                    nc.gpsimd.tensor_reduce(
                        gmax1[0:1, i * gsub + j:i * gsub + j + 1], gx_t,
                        axis=mybir.AxisListType.XYZWC,
                        op=Alu.max, apply_absolute_value=True)
            nc.gpsimd.tensor_reduce(redg1, gmax1,
                                    axis=mybir.AxisListType.XYZWC,
                                    op=Alu.max)

            # ---- beta partial: sum|wb| in 2-ktile strides (ACT abs) ----
            for i in range(WKT // WB2):
                wb_t = wb_pool.tile([128, WB2, O_SH], f32, tag="wbtile",
                                    name="wb_t")
                nc.sync.dma_start(wb_t, wb_r[:, ts(i, WB2), :])
                nc.vector.tensor_reduce(
                    wsum[:, i:i + 1],
                    wb_t.rearrange("p a o -> p (a o)"),
                    axis=mybir.AxisListType.X, op=Alu.add,
                    apply_absolute_value=True)
            nc.vector.tensor_reduce(redw, wsum,
                                    axis=mybir.AxisListType.X, op=Alu.add)

            # ---- fused cross-partition + cross-core reduction ----
            nc.gpsimd.partition_all_reduce(redgw[:, 1:2], redw, channels=128,
                                           reduce_op=bass_isa.ReduceOp.add)
            nc.gpsimd.dma_start(ccx_in[0:1], redg1[0:1, 0:1])
            nc.gpsimd.dma_start(ccx_in[1:2], redgw[0:1, 1:2])
            nc.gpsimd.collective_compute(
                "AllGather", Alu.bypass,
                replica_groups=[list(range(n_cores))],
                ins=[ccx_in.ap()], outs=[ccx_out.ap()])
            nc.gpsimd.dma_start(
                scx1, ccx_out.ap().rearrange("(a b) -> a b", a=1))
            nc.gpsimd.partition_broadcast(scx, scx1)
            scx_v = scx.rearrange("p (c s) -> p s c", s=2)

            # gamma = max over cores; s16 = 16/gamma
            nc.vector.tensor_reduce(scal[:, 0:1], scx_v[:, 0, :],
                                    axis=mybir.AxisListType.X, op=Alu.max)
            nc.vector.tensor_scalar_max(scal[:, 0:1], scal[:, 0:1], EPS)
            nc.vector.reciprocal(n16, scal[:, 0:1])
            nc.vector.tensor_scalar_mul(scal[:, 3:4], n16, 16.0)

            # beta = sum over cores / n_total; thr = beta/2;
            # c_out = beta*gamma/16
            nc.vector.tensor_reduce(scal[:, 1:2], scx_v[:, 1, :],
                                    axis=mybir.AxisListType.X, op=Alu.add)
            inv_n = float(np.float32(1.0) / np.float32(n_total))
            nc.vector.tensor_scalar_mul(scal[:, 2:3], scal[:, 1:2], inv_n)
            nc.vector.tensor_scalar_max(scal[:, 2:3], scal[:, 2:3], EPS)
            nc.vector.tensor_scalar_mul(scal[:, 4:5], scal[:, 2:3], 0.5)
            nc.vector.tensor_tensor(scal[:, 6:7], scal[:, 4:5],
                                    scal[:, 4:5], op=Alu.mult)
            nc.vector.tensor_tensor(scal[:, 5:6], scal[:, 2:3],
                                    scal[:, 0:1], op=Alu.mult)
            nc.vector.tensor_scalar_mul(scal[:, 5:6], scal[:, 5:6],
                                        1.0 / 16.0)

            ab_tiles = {}

            def quantize_block(b, on_act):
                ab8 = ab_pool.tile([128, KT, 2, blk], fp8, name="ab8")
                ab_tiles[b] = ab8
                for qq in range(NQ):
                    x_t = x_pool.tile([128, KQ, blk], f32, tag="x_t",
                                      name="x_t")
                    nc.gpsimd.dma_start(
                        x_t, xT_r[:, ts(qq, KQ), ts(b, blk)])
                    a_sl = ab8[:, ts(qq, KQ), 0, :]
                    if on_act:
                        nc.scalar.activation(x_t, x_t, Act.Copy,
                                             scale=scal[:, 3:4])
                        nc.scalar.activation(a_sl, x_t, Act.Copy)
                    else:
                        nc.vector.tensor_scalar_mul(x_t, x_t,
                                                    scal[:, 3:4])
                        nc.vector.tensor_scalar_mul(a_sl, x_t, 1.0)
                    nc.vector.tensor_tensor(
                        ab8[:, ts(qq, KQ), 1, :], x_t, a_sl,
                        op=Alu.subtract)

            # block 0 quantizes on DVE, ahead of the W stream in DVE
            # program order (gated on gamma only -> PE trails the w8
            # stream through tt0); later blocks go on ACT, pre-emitted
            # a block ahead.
            quantize_block(0, on_act=False)

            # ---- W quantize stream ----
            for k in range(KT):
                wt_t = wt_pool.tile([128, O_SH], f32, tag="wtile",
                                    name="wq_t")
                nc.sync.dma_start(wt_t, wt_r[:, k, :])
                nc.vector.tensor_tensor(wt_t, wt_t, wt_t, op=Alu.mult)
                nc.vector.tensor_scalar(w8[:, k, :], wt_t, scal[:, 6:7],
                                        None, op0=Alu.is_gt)

            def ps_sections(sec_list):
                for (soff, sw) in sec_list:
                    psS = pse_pool.tile([128, 512], f32,
                                        tag="pset", name="psS")
                    for kk in range(KK):
                        nc.tensor.matmul(
                            psS[0:1, 0:sw], ones_lhs,
                            w8[:, 2 * kk:2 * kk + 2, soff:soff + sw],
                            start=(kk == 0), stop=(kk == KK - 1),
                            perf_mode=DR)
                    nc.vector.tensor_scalar_mul(
                        wx8[0:1, 0, soff:soff + sw],
                        psS[0:1, 0:sw], inv_I)

            # ---- main loop ----
            for b in range(NBLK):
                aext = aextA if b % 2 == 0 else aextB
                if b not in ab_tiles:
                    quantize_block(b, on_act=True)
                ab8 = ab_tiles[b]

                for tt in range(TPB):
                    if tt == 1 and 1 <= b + 1 < NBLK \
                            and b + 1 not in ab_tiles:
                        # pre-quantize next block here: its ACT/DVE ops
                        # land between tt0's and tt1's copies in those
                        # FIFOs, hiding under tt1's matmuls
                        quantize_block(b + 1, on_act=True)
                    first = (b == 0 and tt == 0)
                    ph = [ps_pool.tile([128, OH], f32, tag=f"ph{h}",
                                       name=f"ph{h}") for h in range(2)]
                    for kk in range(KK):
                        lhsT = ab8[:, 2 * kk:2 * kk + 2, 0, ts(tt, 128)]
                        for h in range(2):
                            base = h * OH
                            for (off, w_, rs, re) in hchunks:
                                nc.tensor.matmul(
                                    ph[h][:, off:off + w_], lhsT,
                                    w8[:, 2 * kk:2 * kk + 2,
                                       base + off:base + off + w_],
                                    start=(kk == 0 and rs), stop=False,
                                    perf_mode=DR)
                    if tt == 0:
                        # E[t] = sum_k B[t,k] -> aext fp8 row
                        psE = pse_pool.tile([128, 512], f32, tag="pset",
                                            name="psE")
                        for kk in range(KK):
                            nc.tensor.matmul(
                                psE[0:1, 0:blk], ones_lhs,
                                ab8[:, 2 * kk:2 * kk + 2, 1, :],
                                start=(kk == 0), stop=(kk == KK - 1),
                                perf_mode=DR)
                        nc.vector.tensor_scalar_mul(aext[0:1, 0, :],
                                                    psE[0:1, 0:blk], 1.0)

                    for h in range(2):
                        if first:
                            ps_sections(secs_h0 if h == 0 else secs_h1)
                        base = h * OH
                        for (off, w_, rs, re) in hchunks:
                            nc.tensor.matmul(
                                ph[h][:, off:off + w_],
                                aext[:, :, ts(tt, 128)],
                                wx8[:, :, base + off:base + off + w_],
                                start=False, stop=re, perf_mode=DR)
                        y_t = y_pool.tile([128, OH], bf16, tag="yh",
                                          name="y_t")
                        if h == 0:
                            nc.scalar.activation(y_t, ph[h], Act.Copy,
                                                 scale=scal[:, 5:6])
                        else:
                            nc.vector.tensor_scalar(y_t, ph[h],
                                                    scal[:, 5:6], None,
                                                    op0=Alu.mult)
                        nc.gpsimd.dma_start(
                            y_d.ap()[ts(b * TPB + tt, 128),
                                     base:base + OH], y_t)

    nc.compile()
    _dedup_ldweights(nc)
    return nc


def _dedup_ldweights(nc):
    """Drop InstLdweights whose weights AP equals the previous PE load."""
    removed = kept_sync = 0
    for fn in nc.m.functions:
        for blk_ in fn.blocks:
            insts = blk_.instructions
            prev_sig = None
            kill = []
            for j, ins in enumerate(insts):
                tn = type(ins).__name__
                if tn == "InstLdweights":
                    sig = (str(ins.ins[0]), str(ins.perf_mode),
                           str(ins.is_transpose))
                    if sig == prev_sig:
                        if not ins.has_wait() and not ins.has_update():
                            kill.append(j)
                        else:
                            kept_sync += 1
                    prev_sig = sig
                elif tn == "InstMatmult":
                    if ins.is_transpose:
                        prev_sig = None
            for j in reversed(kill):
                del insts[j]
            removed += len(kill)
    if removed:
        print(f"[kernel_sp] deduped {removed} redundant ldweights "
              f"({kept_sync} kept for sync)")


_CACHED_NC = None


def _get_nc():
    global _CACHED_NC
    if _CACHED_NC is None:
        _CACHED_NC = build_kernel()
    return _CACHED_NC


def shard_inputs(x, weight):
    x2 = x.reshape(TOK_TOTAL, I_DIM).astype(np.float32, copy=False)
    weight = weight.astype(np.float32, copy=False)
    xT_halves = [
        np.ascontiguousarray(x2[h * TOK:(h + 1) * TOK].T)
        for h in range(TOK_HALVES)
    ]
    wt_quarters = [
        np.ascontiguousarray(weight[q * O_SH:(q + 1) * O_SH].T)
        for q in range(O_QUARTERS)
    ]
    gk = I_DIM // O_QUARTERS
    bk = I_DIM // TOK_HALVES
    in_maps = []
    for c in range(N_CORES):
        h, q = c // O_QUARTERS, c % O_QUARTERS
        in_maps.append({
            "xT": xT_halves[h],
            "wt": wt_quarters[q],
            "xg": np.ascontiguousarray(xT_halves[h][q * gk:(q + 1) * gk]),
            "wb": np.ascontiguousarray(wt_quarters[q][h * bk:(h + 1) * bk]),
        })
    return in_maps


def unshard_output(results):
    rows = []
    for h in range(TOK_HALVES):
        cols = [np.asarray(results[h * O_QUARTERS + q]["y"])
                for q in range(O_QUARTERS)]
        rows.append(np.concatenate(cols, axis=1))
    y = np.concatenate(rows, axis=0).astype(np.float32)
    return y.reshape(B_DIM, S_DIM, O_DIM)


def run_on_cores(x, weight, trace=False):
    from concourse.bass_utils import run_bass_kernel_spmd
    nc = _get_nc()
    in_maps = shard_inputs(x, weight)
    res = run_bass_kernel_spmd(nc, in_maps, core_ids=list(range(N_CORES)),
                               trace=trace)
    return res


def kernel(x, weight):
    res = run_on_cores(x, weight, trace=False)
    return unshard_output(res.results)
